# revision 21
# baseline (speedup 1.0000x reference)
"""Paged prefill attention (sparse_attention) on 8 Trainium2 NeuronCores.

Problem (hardcoded, mirrors the reference):
  q:        [2048, 32, 128] f32   (2 seqs x 1024 query tokens, 32 heads)
  k_cache:  [64, 64, 8, 128] f32  (64 physical blocks x 64 tokens x 8 kv heads)
  v_cache:  [64, 64, 8, 128] f32
  cu_seqlens_q: [0, 1024, 2048]
  cu_seqlens_k: [0, 2048, 4096]
  block_tables: [2, 32] int32 permutation of the 64 physical blocks
  out:      [2048, 32, 128] f32

Sharding: tensor-parallel by kv head. Core h gets kv head h plus its 4
query heads (GQA group 4), both full sequences. One static program runs
SPMD on all 8 cores. Input marshaling (per-core slice, fp16 cast,
[d, token] transposes, block-table ordering of the KV slices) happens on
the host while building each core's input arrays — the device program is
block-table independent.

Per-core device program (S^T layout flash attention, fp16 matmuls),
software-pipelined across all 8 (seq, head) problems:
  - qT [128 d, 8192 (s,h,t)] f16, kT [128 d, 4096 (s,t)] f16 and
    vP [128 tok, 32 chunks x 129] f16 (col 128 of each chunk = ones, the
    softmax denominator accumulator) land via split input DMAs.
  - QK S^T[k,q] per 128-token chunk into a 2-bank PSUM region
    (ping-pong, causal-clipped per chunk).
  - softmax exp: mostly on ScalarE (PSUM -> fp16 es tiles); a subset of
    history chunks (OFFLOAD) instead uses a two-pass fp16 Schraudolph
    bit-trick on the otherwise-idle VectorE: es_bits =
    int16(score * A + B) reinterpreted as fp16 ~= exp(scale * score)
    within +-3.6%, rebalancing the ScalarE bottleneck.
  - diagonal chunks: strictly-below-diagonal es zeroed by GPSIMD
    affine_select (off the PE/ACT critical chain).
  - PV accumulates es.T @ vP into a PSUM-resident [128, 129]-per-qt
    block (3 banks, one accumulation group per bank), lagging QK by
    LAG chunks globally (cross-head pipelining, no boundary bubbles).
  - Drain per bank group: one DVE tensor_scalar divide straight from
    PSUM (unnormalized out / ones-column denominator) into the staged
    output, then a per-group DMA out.
"""

import numpy as np

NUM_SEQS = 2
LQ = 1024
HIST = 1024
LK = LQ + HIST
NUM_HEADS = 32
NUM_KV_HEADS = 8
GROUP = NUM_HEADS // NUM_KV_HEADS  # 4 q heads per kv head / core
HEAD_DIM = 128
BLOCK_SIZE = 64
NBLK = LK // BLOCK_SIZE         # 32 logical blocks per sequence
TOTAL_BLOCKS = NUM_SEQS * NBLK  # 64 physical blocks
NCH = LK // 128                 # 16 128-token kv chunks per sequence
NQT = LQ // 128                 # 8 128-token q tiles per sequence
SCALE = 1.0 / float(np.sqrt(HEAD_DIM))

NTOK = NUM_SEQS * LK            # 4096 kv tokens
NQCOL = NUM_SEQS * LQ * GROUP   # 8192 qT columns

LAG = 5                         # PV chunks behind QK in the global pipeline
OFFLOAD = frozenset({3, 6})     # per-head history chunks exp'd on VectorE
EXPA = 130.57784916438905       # SCALE * log2(e) * 1024
EXPB = 15308.0                  # 15 * 1024 - 52 (calibrated vs HW rint)

_CACHE = {}


def _po_slot(qt):
    # po banks hold qt {0,1,2}, {3,4,5}, {6,7}: 129 f32 slots, bank-local
    return (qt // 3) * 512 + (qt % 3) * 129


_DRAIN = {10: (0, 1, 2), 13: (3, 4, 5), 15: (6, 7)}


def _build_program():
    from contextlib import ExitStack

    import concourse.mybir as mybir
    import concourse.tile as tile
    from concourse import bacc

    f32 = mybir.dt.float32
    f16 = mybir.dt.float16
    i16 = mybir.dt.int16

    nc = bacc.Bacc()
    qT_d = nc.dram_tensor("qT", [HEAD_DIM, NQCOL], f16, kind="ExternalInput")
    kT_d = nc.dram_tensor("kT", [HEAD_DIM, NTOK], f16, kind="ExternalInput")
    vP_d = nc.dram_tensor("vP", [128, NUM_SEQS * NCH * 129], f16,
                          kind="ExternalInput")
    o_d = nc.dram_tensor("out", [NUM_SEQS * LQ, GROUP, HEAD_DIM], f32,
                         kind="ExternalOutput")

    with tile.TileContext(nc) as tc, ExitStack() as ctx:
        persist = ctx.enter_context(tc.tile_pool(name="persist", bufs=1))
        es_pool = ctx.enter_context(tc.tile_pool(name="es", bufs=18))
        scr_pool = ctx.enter_context(tc.tile_pool(name="scr", bufs=4))
        ost_pool = ctx.enter_context(tc.tile_pool(name="ost", bufs=2))
        ob_pool = ctx.enter_context(tc.tile_pool(name="ob", bufs=6))
        qk_ps = ctx.enter_context(tc.tile_pool(name="qk_ps", bufs=3,
                                               space="PSUM"))
        po_ps = ctx.enter_context(tc.tile_pool(name="po_ps", bufs=2,
                                               space="PSUM"))

        kT = persist.tile([128, NTOK], f16, tag="kT")
        qT = persist.tile([128, NQCOL], f16, tag="qT")
        vP = persist.tile([128, NUM_SEQS * NCH * 129], f16, tag="vP")

        # split input DMAs, startup-criticality order
        VH = NCH * 129
        nc.sync.dma_start(out=qT[:, 0:LQ], in_=qT_d[:, 0:LQ])
        nc.sync.dma_start(out=kT[:, 0:128], in_=kT_d[:, 0:128])
        nc.sync.dma_start(out=kT[:, 128:1024], in_=kT_d[:, 128:1024])
        nc.sync.dma_start(out=vP[:, 0:8 * 129], in_=vP_d[:, 0:8 * 129])
        nc.sync.dma_start(out=kT[:, 1024:LK], in_=kT_d[:, 1024:LK])
        nc.sync.dma_start(out=vP[:, 8 * 129:VH], in_=vP_d[:, 8 * 129:VH])
        nc.sync.dma_start(out=qT[:, LQ:4 * LQ], in_=qT_d[:, LQ:4 * LQ])
        nc.sync.dma_start(out=kT[:, LK:NTOK], in_=kT_d[:, LK:NTOK])
        nc.sync.dma_start(out=vP[:, VH:2 * VH], in_=vP_d[:, VH:2 * VH])
        nc.sync.dma_start(out=qT[:, 4 * LQ:NQCOL], in_=qT_d[:, 4 * LQ:NQCOL])

        heads = [(s, h) for s in range(NUM_SEQS) for h in range(GROUP)]
        state = {}  # hi -> dict(po=, ost=, es=)

        def emit_qk(hi, c):
            s, h = heads[hi]
            qbase = (s * GROUP + h) * LQ
            q_lo = max(0, (c - 8) * 128)
            width = LQ - q_lo
            ps = qk_ps.tile([128, 1024], f32, tag="qk")
            for off in range(0, width, 512):
                n = min(512, width - off)
                nc.tensor.matmul(
                    ps[:, off:off + n],
                    kT[:, s * LK + c * 128:s * LK + (c + 1) * 128],
                    qT[:, qbase + q_lo + off:qbase + q_lo + off + n],
                    start=True, stop=True)
            es = es_pool.tile([128, 1024], f16, tag="es")
            if c in OFFLOAD:
                # fp16 Schraudolph exp on VectorE (history chunks only)
                scr = scr_pool.tile([128, 1024], f32, tag="scr")
                nc.vector.tensor_scalar(
                    out=scr[:, 0:width], in0=ps[:, 0:width],
                    scalar1=EXPA, scalar2=EXPB,
                    op0=mybir.AluOpType.mult, op1=mybir.AluOpType.add)
                nc.vector.tensor_copy(
                    out=es[:, 0:width].bitcast(i16), in_=scr[:, 0:width])
            else:
                nc.scalar.activation(
                    es[:, 0:width], ps[:, 0:width],
                    mybir.ActivationFunctionType.Exp, scale=SCALE)
            if c >= 8:
                # zero strictly-below-diagonal of the in-chunk diag block
                nc.gpsimd.affine_select(
                    out=es[:, 0:128], in_=es[:, 0:128],
                    compare_op=mybir.AluOpType.is_ge, fill=0.0,
                    base=0, pattern=[[1, 128]], channel_multiplier=-1)
            state[hi]["es"][c] = es

        def drain(hi, wave, qts):
            s, h = heads[hi]
            st = state[hi]
            po, ost = st["po"][wave], st["ost"]
            for qt in qts:
                sl = (qt - qts[0]) * 129
                rc = ob_pool.tile([128, 1], f32, tag="rc", name="rc")
                nc.vector.reciprocal(rc[:, :], po[:, sl + 128:sl + 129])
                nc.vector.tensor_scalar_mul(
                    ost[:, qt * 128:(qt + 1) * 128],
                    po[:, sl:sl + 128], rc[:, :])
                if qt != qts[-1] and not (hi == len(heads) - 1 and wave == 2):
                    continue
                # one DMA per drained group; per-qt for the last head's
                # final wave to shorten the tail
                r0 = qt * 128 if (hi == len(heads) - 1 and wave == 2) \
                    else qts[0] * 128
                r1 = (qt + 1) * 128
                o_view = o_d[s * LQ + r0:s * LQ + r1, h, :].rearrange(
                    "(c p) d -> p c d", p=128)
                nc.sync.dma_start(
                    out=o_view,
                    in_=ost[:, r0:r1].rearrange("p (c d) -> p c d", d=128))

        def wave_layout(hi):
            # (wave_idx, qts) for the two inline waves, plus the queued
            # wave that reuses wave-0's bank after its c==10 drain.  The
            # last head keeps {6,7} inline so after the final exp only
            # one PV matmul remains before the last drain+DMA.
            if hi == len(heads) - 1:
                return [(0, (0, 1, 2)), (1, (6, 7))], (2, (3, 4, 5))
            return [(0, (0, 1, 2)), (1, (3, 4, 5))], (2, (6, 7))

        def pv_mm(st, wave, qts, qt, c, s):
            po = st["po"][wave]
            q_lo = max(0, (c - 8) * 128)
            es = st["es"][c]
            sl = (qt - qts[0]) * 129
            nc.tensor.matmul(
                po[:, sl:sl + 129],
                es[:, qt * 128 - q_lo:qt * 128 - q_lo + 128],
                vP[:, (s * NCH + c) * 129:(s * NCH + c + 1) * 129],
                start=(c == 0 and qt == qts[0]),
                stop=(c == qts[-1] + 8 and qt == qts[-1]))

        def emit_pv2(hi, c):
            s, h = heads[hi]
            st = state[hi]
            _, (wave, qts) = wave_layout(hi)
            if st["po"][wave] is None:
                st["po"][wave] = po_ps.tile([128, 512], f32, tag="po",
                                            name="po2")
            for qt in qts:
                if c - 8 <= qt and c <= qt + 8:
                    pv_mm(st, wave, qts, qt, c, s)
            st["es"].pop(c, None)
            if c == qts[-1] + 8:
                drain(hi, wave, qts)

        def emit_pv(hi, c):
            s, h = heads[hi]
            st = state[hi]
            inline, (w2, qts2) = wave_layout(hi)
            if c == 0:
                st["po"][inline[0][0]] = po_ps.tile(
                    [128, 512], f32, tag="po", name="po0")
                st["po"][inline[1][0]] = po_ps.tile(
                    [128, 512], f32, tag="po", name="po1")
                st["ost"] = ost_pool.tile([128, LQ], f32, tag="ost",
                                          name="ost")
            for wave, qts in inline:
                for qt in qts:
                    if c - 8 <= qt and c <= qt + 8:
                        pv_mm(st, wave, qts, qt, c, s)
                if c == qts[-1] + 8:
                    drain(hi, wave, qts)
            last = hi == len(heads) - 1
            if c <= qts2[-1] + 8:
                st["w2q"].append(c)
            else:
                st["es"].pop(c, None)
            if c >= 10:
                for _ in range(5 if last else (4 if c > 10 else 0)):
                    if st["w2q"]:
                        emit_pv2(hi, st["w2q"].popleft())
            if c == NCH - 1:
                while st["w2q"]:
                    emit_pv2(hi, st["w2q"].popleft())

        from collections import deque
        ops = [(hi, c) for hi in range(len(heads)) for c in range(NCH)]
        for hi in range(len(heads)):
            state[hi] = {"po": [None, None, None], "ost": None,
                         "es": {}, "w2q": deque()}
        n_ops = len(ops)
        pv_ptr = 0
        for i, (hi, c) in enumerate(ops):
            emit_qk(hi, c)
            lag = 1 if hi == len(heads) - 1 else LAG
            while pv_ptr <= i - lag:
                emit_pv(*ops[pv_ptr])
                pv_ptr += 1
        while pv_ptr < n_ops:
            emit_pv(*ops[pv_ptr])
            pv_ptr += 1

    nc.compile()
    return nc


def _get_program():
    if "prog" not in _CACHE:
        _CACHE["prog"] = _build_program()
    return _CACHE["prog"]


def _marshal_core(q, k_cache, v_cache, rows, core):
    """Build one core's input arrays: fp16, transposed, block-table order."""
    q16 = np.ascontiguousarray(
        q[:, core * GROUP:(core + 1) * GROUP, :]).astype(np.float16)
    # qT[d, s*4096 + h*1024 + t] = q[s*1024 + t, h, d]
    qT = np.ascontiguousarray(
        q16.reshape(NUM_SEQS, LQ, GROUP, HEAD_DIM)
        .transpose(3, 0, 2, 1).reshape(HEAD_DIM, NQCOL))

    k16 = k_cache[:, :, core, :].reshape(NTOK, HEAD_DIM).astype(np.float16)
    v16 = v_cache[:, :, core, :].reshape(NTOK, HEAD_DIM).astype(np.float16)
    kT = np.ascontiguousarray(k16[rows].T)           # [128, 4096]

    vl = v16[rows].reshape(NUM_SEQS * NCH, 128, HEAD_DIM)
    vP = np.ones((128, NUM_SEQS * NCH, 129), dtype=np.float16)
    vP[:, :, 0:HEAD_DIM] = vl.transpose(1, 0, 2)
    return {"qT": qT, "kT": kT,
            "vP": np.ascontiguousarray(vP.reshape(128, NUM_SEQS * NCH * 129))}


def kernel(q, k_cache, v_cache, cu_seqlens_q, cu_seqlens_k, block_tables,
           _want_trace=False):
    from concourse import bass_utils

    q = np.asarray(q, dtype=np.float32)
    k_cache = np.asarray(k_cache, dtype=np.float32)
    v_cache = np.asarray(v_cache, dtype=np.float32)
    bt = np.asarray(block_tables, dtype=np.int32)

    assert q.shape == (NUM_SEQS * LQ, NUM_HEADS, HEAD_DIM)
    assert k_cache.shape == (TOTAL_BLOCKS, BLOCK_SIZE, NUM_KV_HEADS, HEAD_DIM)
    assert v_cache.shape == (TOTAL_BLOCKS, BLOCK_SIZE, NUM_KV_HEADS, HEAD_DIM)
    assert bt.shape == (NUM_SEQS, NBLK)
    assert bt.min() >= 0

    nc = _get_program()

    # DRAM row of logical kv token (s, t): block-table gather order
    t = np.arange(LK, dtype=np.int64)
    rows = np.concatenate(
        [bt[s, t // BLOCK_SIZE] * BLOCK_SIZE + t % BLOCK_SIZE
         for s in range(NUM_SEQS)])

    in_maps = [_marshal_core(q, k_cache, v_cache, rows, core)
               for core in range(NUM_KV_HEADS)]

    res = bass_utils.run_bass_kernel_spmd(
        nc, in_maps, core_ids=list(range(NUM_KV_HEADS)),
        trace=_want_trace,
        **({"trace_cores": list(range(NUM_KV_HEADS)), "stitch_traces": True}
           if _want_trace else {}),
    )

    out = np.empty((NUM_SEQS * LQ, NUM_HEADS, HEAD_DIM), dtype=np.float32)
    for core in range(NUM_KV_HEADS):
        out[:, core * GROUP:(core + 1) * GROUP, :] = res.results[core]["out"]

    if _want_trace:
        return out, res
    return out


# revision 46
# speedup vs baseline: 1.1336x; 1.1336x over previous
"""Paged prefill attention (sparse_attention) on 8 Trainium2 NeuronCores.

Problem (hardcoded, mirrors the reference):
  q:        [2048, 32, 128] f32   (2 seqs x 1024 query tokens, 32 heads)
  k_cache:  [64, 64, 8, 128] f32  (64 physical blocks x 64 tokens x 8 kv heads)
  v_cache:  [64, 64, 8, 128] f32
  cu_seqlens_q: [0, 1024, 2048]
  cu_seqlens_k: [0, 2048, 4096]
  block_tables: [2, 32] int32 permutation of the 64 physical blocks
  out:      [2048, 32, 128] f32

Sharding: tensor-parallel by kv head. Core h gets kv head h plus its 4
query heads (GQA group 4), both full sequences. One static program runs
SPMD on all 8 cores. Input marshaling (per-core slice, fp16 cast,
[d, token] transposes, block-table ordering of the KV slices) happens on
the host while building each core's input arrays — the device program is
block-table independent.

Per-core device program (S^T layout flash attention, fp16 matmuls),
software-pipelined across all 8 (seq, head) problems:
  - qT [128 d, 8192 (s,h,t)] f16, kT [128 d, 4096 (s,t)] f16 and
    vP [128 tok, 32 chunks x 129] f16 (col 128 of each chunk = ones, the
    softmax denominator accumulator) land via split input DMAs.
  - QK S^T[k,q] per 128-token chunk into a 2-bank PSUM region
    (ping-pong, causal-clipped per chunk).
  - softmax exp: mostly on ScalarE (PSUM -> fp16 es tiles), with the
    small diagonal chunks (12,13) and (14,15) packed pairwise into one
    region/exp call each.  Two history chunks per head (OFFLOAD) instead
    use a fp16 Schraudolph bit-trick off the ScalarE bottleneck:
    VectorE computes t = score * A + B (f32), GPSIMD converts t to int16
    whose bits reinterpret as fp16 ~= exp(scale * score) within ~3%,
    with A, B calibrated against the HW round-to-nearest conversion.
  - diagonal chunks: strictly-below-diagonal es zeroed by GPSIMD
    affine_select (off the PE/ACT critical chain).
  - PV accumulates es.T @ vP into PSUM-resident [128, 129]-per-qt
    slots in two banks via three waves per head (qt {0,1,2} and {3,4,5}
    concurrently, {6,7} reusing the first bank after its drain), one
    accumulation group per bank.  PV lags QK by LAG chunks globally
    (cross-head pipelining), leaving six PSUM banks for three QK
    regions so the PE runs far enough ahead to hide the offload gaps.
  - Drain per bank group: one DVE tensor_scalar divide straight from
    PSUM (unnormalized out / ones-column denominator) into the staged
    output, then a per-group DMA out.
"""

import numpy as np

NUM_SEQS = 2
LQ = 1024
HIST = 1024
LK = LQ + HIST
NUM_HEADS = 32
NUM_KV_HEADS = 8
GROUP = NUM_HEADS // NUM_KV_HEADS  # 4 q heads per kv head / core
HEAD_DIM = 128
BLOCK_SIZE = 64
NBLK = LK // BLOCK_SIZE         # 32 logical blocks per sequence
TOTAL_BLOCKS = NUM_SEQS * NBLK  # 64 physical blocks
NCH = LK // 128                 # 16 128-token kv chunks per sequence
NQT = LQ // 128                 # 8 128-token q tiles per sequence
SCALE = 1.0 / float(np.sqrt(HEAD_DIM))

NTOK = NUM_SEQS * LK            # 4096 kv tokens
NQCOL = NUM_SEQS * LQ * GROUP   # 8192 qT columns

import os
LAG = int(os.environ.get("K_LAG", "6"))
K_LAYOUT_LAST = os.environ.get("K_LAYOUT_LAST", "A")
K_LAG_LAST = int(os.environ.get("K_LAG_LAST", "0"))
K_DRIP_LAST = int(os.environ.get("K_DRIP_LAST", "0"))
K_DRIP = int(os.environ.get("K_DRIP", "1"))
OFFLOAD = frozenset(int(x) for x in os.environ.get("K_OFF", "3,6").split(","))
EXPA = 130.57784916438905       # SCALE * log2(e) * 1024
EXPB = 15308.0                  # 15 * 1024 - 52 (calibrated vs HW rint)

_CACHE = {}


def _po_slot(qt):
    # po banks hold qt {0,1,2}, {3,4,5}, {6,7}: 129 f32 slots, bank-local
    return (qt // 3) * 512 + (qt % 3) * 129


_DRAIN = {10: (0, 1, 2), 13: (3, 4, 5), 15: (6, 7)}


def _build_program():
    from contextlib import ExitStack

    import concourse.mybir as mybir
    import concourse.tile as tile
    from concourse import bacc

    f32 = mybir.dt.float32
    f16 = mybir.dt.float16
    i16 = mybir.dt.int16

    nc = bacc.Bacc()
    qT_d = nc.dram_tensor("qT", [HEAD_DIM, NQCOL], f16, kind="ExternalInput")
    kT_d = nc.dram_tensor("kT", [HEAD_DIM, NTOK], f16, kind="ExternalInput")
    vP_d = nc.dram_tensor("vP", [128, NUM_SEQS * NCH * 129], f16,
                          kind="ExternalInput")
    o_d = nc.dram_tensor("out", [NUM_SEQS * LQ, GROUP, HEAD_DIM], f32,
                         kind="ExternalOutput")

    with tile.TileContext(nc) as tc, ExitStack() as ctx:
        persist = ctx.enter_context(tc.tile_pool(name="persist", bufs=1))
        es_pool = ctx.enter_context(tc.tile_pool(name="es", bufs=18))
        scr_pool = ctx.enter_context(tc.tile_pool(name="scr", bufs=int(_os.environ.get("K_SCRB", "4"))))
        ost_pool = ctx.enter_context(tc.tile_pool(name="ost", bufs=int(_os.environ.get("K_OSTB", "2"))))
        ob_pool = ctx.enter_context(tc.tile_pool(name="ob", bufs=6))
        qk_ps = ctx.enter_context(tc.tile_pool(name="qk_ps", bufs=3,
                                               space="PSUM"))
        po_ps = ctx.enter_context(tc.tile_pool(name="po_ps", bufs=2,
                                               space="PSUM"))

        kT = persist.tile([128, NTOK], f16, tag="kT")
        qT = persist.tile([128, NQCOL], f16, tag="qT")
        vP = persist.tile([128, NUM_SEQS * NCH * 129], f16, tag="vP")

        # split input DMAs, startup-criticality order
        VH = NCH * 129
        dma_order = os.environ.get("K_DMA_ORDER", "D")
        if dma_order == "G":
            pieces = [("k", 0, 128), ("q", 0, 512), ("q", 512, LQ),
                      ("k", 128, 1024), ("v", 0, 1032), ("k", 1024, LK),
                      ("v", 1032, VH), ("q", LQ, 4 * LQ), ("k", LK, NTOK),
                      ("v", VH, 2 * VH), ("q", 4 * LQ, NQCOL)]
        elif dma_order == "D":
            pieces = [("k", 0, 128), ("q", 0, LQ), ("k", 128, 1024),
                      ("v", 0, 1032), ("k", 1024, LK), ("v", 1032, VH),
                      ("q", LQ, 4 * LQ), ("k", LK, NTOK),
                      ("v", VH, 2 * VH), ("q", 4 * LQ, NQCOL)]
        elif dma_order == "E":
            pieces = [("k", 0, 128), ("q", 0, LQ), ("v", 0, 1032),
                      ("k", 128, 1024), ("k", 1024, LK), ("v", 1032, VH),
                      ("q", LQ, 4 * LQ), ("k", LK, NTOK),
                      ("v", VH, 2 * VH), ("q", 4 * LQ, NQCOL)]
        elif dma_order == "F":
            pieces = [("k", 0, 128), ("q", 0, 2 * LQ), ("k", 128, 1024),
                      ("v", 0, 1032), ("k", 1024, LK), ("v", 1032, VH),
                      ("q", 2 * LQ, 4 * LQ), ("k", LK, NTOK),
                      ("v", VH, 2 * VH), ("q", 4 * LQ, NQCOL)]
        elif dma_order == "A":
            pieces = [("q", 0, LQ), ("k", 0, 128), ("k", 128, 1024),
                      ("v", 0, 1032), ("k", 1024, LK), ("v", 1032, VH),
                      ("q", LQ, 4 * LQ), ("k", LK, NTOK),
                      ("v", VH, 2 * VH), ("q", 4 * LQ, NQCOL)]
        elif dma_order == "B":
            pieces = [("q", 0, LQ), ("k", 0, 1024), ("v", 0, 1032),
                      ("k", 1024, LK), ("v", 1032, VH),
                      ("q", LQ, 4 * LQ), ("k", LK, NTOK),
                      ("v", VH, 2 * VH), ("q", 4 * LQ, NQCOL)]
        else:
            pieces = [("q", 0, LQ), ("kp", 0, 128), ("kp", 128, 1024),
                      ("v", 0, 1032), ("k", 1024, LK), ("v", 1032, VH),
                      ("q", LQ, 4 * LQ), ("k", LK, NTOK),
                      ("v", VH, 2 * VH), ("q", 4 * LQ, NQCOL)]
        srcs = {"q": (qT, qT_d), "k": (kT, kT_d), "v": (vP, vP_d)}
        for t, a, b in pieces:
            dst, sd = srcs[t[0]]
            eng = nc.gpsimd if t.endswith("p") else nc.sync
            eng.dma_start(out=dst[:, a:b], in_=sd[:, a:b])

        heads = [(s, h) for s in range(NUM_SEQS) for h in range(GROUP)]
        state = {}  # hi -> dict(po=, ost=, es=)

        def emit_qk_group(hi, group):
            # one PSUM region + one exp call for a group of chunks
            s, h = heads[hi]
            if hi == 0 and group == [0] and os.environ.get("K_SPLIT0"):
                # pipeline-fill: split the very first chunk into two
                # 512-wide QK+exp halves so ScalarE starts sooner
                ps = qk_ps.tile([128, 1024], f32, tag="qk")
                es = es_pool.tile([128, 1024], f16, tag="es")
                for half in range(2):
                    o = half * 512
                    nc.tensor.matmul(
                        ps[:, o:o + 512], kT[:, 0:128],
                        qT[:, o:o + 512], start=True, stop=True)
                    nc.scalar.activation(
                        es[:, o:o + 512], ps[:, o:o + 512],
                        mybir.ActivationFunctionType.Exp, scale=SCALE)
                state[0]["es"][0] = (es, 0)
                return
            qbase = (s * GROUP + h) * LQ
            ps = qk_ps.tile([128, 1024], f32, tag="qk")
            offs, off = {}, 0
            for c in group:
                offs[c] = off
                off += LQ - max(0, (c - 8) * 128)
            total = off
            same_bank = len(group) > 1 and total <= 512
            for gi, c in enumerate(group):
                q_lo = max(0, (c - 8) * 128)
                width = LQ - q_lo
                base = offs[c]
                for mo in range(0, width, 512):
                    n = min(512, width - mo)
                    if same_bank:
                        st_flag = gi == 0
                        sp_flag = gi == len(group) - 1
                    else:
                        st_flag = sp_flag = True
                    nc.tensor.matmul(
                        ps[:, base + mo:base + mo + n],
                        kT[:, s * LK + c * 128:s * LK + (c + 1) * 128],
                        qT[:, qbase + q_lo + mo:qbase + q_lo + mo + n],
                        start=st_flag, stop=sp_flag)
            es = es_pool.tile([128, 1024], f16, tag="es")
            c0 = group[0]
            if c0 in OFFLOAD:
                assert len(group) == 1
                scr = scr_pool.tile([128, 1024], f32, tag="scr")
                nc.vector.tensor_scalar(
                    out=scr[:, 0:total], in0=ps[:, 0:total],
                    scalar1=EXPA, scalar2=EXPB,
                    op0=mybir.AluOpType.mult, op1=mybir.AluOpType.add)
                eng = nc.vector if os.environ.get("K_PASS2_DVE") \
                    else nc.gpsimd
                eng.tensor_copy(
                    out=es[:, 0:total].bitcast(i16), in_=scr[:, 0:total])
            else:
                nc.scalar.activation(
                    es[:, 0:total], ps[:, 0:total],
                    mybir.ActivationFunctionType.Exp, scale=SCALE)
            for c in group:
                if c >= 8:
                    nc.gpsimd.affine_select(
                        out=es[:, offs[c]:offs[c] + 128],
                        in_=es[:, offs[c]:offs[c] + 128],
                        compare_op=mybir.AluOpType.is_ge, fill=0.0,
                        base=0, pattern=[[1, 128]], channel_multiplier=-1)
                state[hi]["es"][c] = (es, offs[c])

        def drain(hi, wave, qts):
            s, h = heads[hi]
            st = state[hi]
            po, ost = st["po"][wave], st["ost"]
            if hi == len(heads) - 1 and wave == 2:
                qts = tuple(reversed(qts))
            base_qt = min(qts)
            for qt in qts:
                sl = (qt - base_qt) * 129
                rc = ob_pool.tile([128, 1], f32, tag="rc", name="rc")
                nc.vector.reciprocal(rc[:, :], po[:, sl + 128:sl + 129])
                nc.vector.tensor_scalar_mul(
                    ost[:, qt * 128:(qt + 1) * 128],
                    po[:, sl:sl + 128], rc[:, :])
                perqt = hi == len(heads) - 1 and wave == 2
                if qt != qts[-1] and not perqt:
                    continue
                # one DMA per drained group; per-qt for the last head's
                # final wave to shorten the tail
                r0 = qt * 128 if perqt else qts[0] * 128
                r1 = (qt + 1) * 128
                o_view = o_d[s * LQ + r0:s * LQ + r1, h, :].rearrange(
                    "(c p) d -> p c d", p=128)
                nc.sync.dma_start(
                    out=o_view,
                    in_=ost[:, r0:r1].rearrange("p (c d) -> p c d", d=128))

        def wave_layout(hi):
            # (wave_idx, qts) for the two inline waves, plus the queued
            # wave that reuses wave-0's bank after its c==10 drain.  The
            # last head keeps {6,7} inline so after the final exp only
            # one PV matmul remains before the last drain+DMA.
            if hi == len(heads) - 1 and K_LAYOUT_LAST == "B":
                return [(0, (0, 1, 2)), (1, (6, 7))], (2, (3, 4, 5))
            return [(0, (0, 1, 2)), (1, (3, 4, 5))], (2, (6, 7))

        def pv_mm(st, wave, qts, qt, c, s):
            po = st["po"][wave]
            q_lo = max(0, (c - 8) * 128)
            es, base = st["es"][c]
            sl = (qt - qts[0]) * 129
            col = base + qt * 128 - q_lo
            nc.tensor.matmul(
                po[:, sl:sl + 129],
                es[:, col:col + 128],
                vP[:, (s * NCH + c) * 129:(s * NCH + c + 1) * 129],
                start=(c == 0 and qt == qts[0]),
                stop=(c == qts[-1] + 8 and qt == qts[-1]))

        def emit_pv2(hi, c):
            s, h = heads[hi]
            st = state[hi]
            _, (wave, qts) = wave_layout(hi)
            if st["po"][wave] is None:
                st["po"][wave] = po_ps.tile([128, 512], f32, tag="po",
                                            name="po2")
            for qt in qts:
                if c - 8 <= qt and c <= qt + 8:
                    pv_mm(st, wave, qts, qt, c, s)
            st["es"].pop(c, None)
            if c == qts[-1] + 8:
                drain(hi, wave, qts)

        def emit_pv(hi, c):
            s, h = heads[hi]
            st = state[hi]
            inline, (w2, qts2) = wave_layout(hi)
            if c == 0:
                st["po"][inline[0][0]] = po_ps.tile(
                    [128, 512], f32, tag="po", name="po0")
                st["po"][inline[1][0]] = po_ps.tile(
                    [128, 512], f32, tag="po", name="po1")
                st["ost"] = ost_pool.tile([128, LQ], f32, tag="ost",
                                          name="ost")
            for wave, qts in inline:
                for qt in qts:
                    if c - 8 <= qt and c <= qt + 8:
                        pv_mm(st, wave, qts, qt, c, s)
                if c == qts[-1] + 8:
                    drain(hi, wave, qts)
            last = hi == len(heads) - 1
            if last and c == 11:
                for _ in range(int(os.environ.get("K_LASTFLUSH", "0"))):
                    if st["w2q"]:
                        emit_pv2(hi, st["w2q"].popleft())
            if c <= qts2[-1] + 8:
                st["w2q"].append(c)
            else:
                st["es"].pop(c, None)
            if c >= 10:
                if last and c == 10 and os.environ.get("K_BURST10"):
                    while st["w2q"]:
                        emit_pv2(hi, st["w2q"].popleft())
                n = K_DRIP_LAST if last else (K_DRIP if c > 10 else 0)
                for _ in range(n):
                    if st["w2q"]:
                        emit_pv2(hi, st["w2q"].popleft())
            if c == NCH - 1:
                while st["w2q"]:
                    emit_pv2(hi, st["w2q"].popleft())

        from collections import deque
        groups = [[0], [1], [2], [3], [4], [5], [6], [7], [8], [9], [10],
                  [11], [12, 13], [14, 15]]
        qk_ops = [(hi, g) for hi in range(len(heads)) for g in groups]
        chunk_seq = [(hi, c) for hi in range(len(heads)) for c in range(NCH)]
        for hi in range(len(heads)):
            state[hi] = {"po": [None, None, None], "ost": None,
                         "es": {}, "w2q": deque()}
        n_chunks = len(chunk_seq)
        pv_ptr = 0
        done = 0
        for hi, g in qk_ops:
            emit_qk_group(hi, g)
            done += len(g)
            if K_LAG_LAST == 0:
                lag = max(1, min(LAG, n_chunks - done))
            else:
                lag = K_LAG_LAST if hi == len(heads) - 1 else LAG
            while pv_ptr <= done - 1 - lag:
                emit_pv(*chunk_seq[pv_ptr])
                pv_ptr += 1
        while pv_ptr < n_chunks:
            emit_pv(*chunk_seq[pv_ptr])
            pv_ptr += 1

    nc.compile()
    return nc


def _get_program():
    if "prog" not in _CACHE:
        _CACHE["prog"] = _build_program()
    return _CACHE["prog"]


def _marshal_core(q, k_cache, v_cache, rows, core):
    """Build one core's input arrays: fp16, transposed, block-table order."""
    q16 = np.ascontiguousarray(
        q[:, core * GROUP:(core + 1) * GROUP, :]).astype(np.float16)
    # qT[d, s*4096 + h*1024 + t] = q[s*1024 + t, h, d]
    qT = np.ascontiguousarray(
        q16.reshape(NUM_SEQS, LQ, GROUP, HEAD_DIM)
        .transpose(3, 0, 2, 1).reshape(HEAD_DIM, NQCOL))

    k16 = k_cache[:, :, core, :].reshape(NTOK, HEAD_DIM).astype(np.float16)
    v16 = v_cache[:, :, core, :].reshape(NTOK, HEAD_DIM).astype(np.float16)
    kT = np.ascontiguousarray(k16[rows].T)           # [128, 4096]

    vl = v16[rows].reshape(NUM_SEQS * NCH, 128, HEAD_DIM)
    vP = np.ones((128, NUM_SEQS * NCH, 129), dtype=np.float16)
    vP[:, :, 0:HEAD_DIM] = vl.transpose(1, 0, 2)
    return {"qT": qT, "kT": kT,
            "vP": np.ascontiguousarray(vP.reshape(128, NUM_SEQS * NCH * 129))}


def kernel(q, k_cache, v_cache, cu_seqlens_q, cu_seqlens_k, block_tables,
           _want_trace=False):
    from concourse import bass_utils

    q = np.asarray(q, dtype=np.float32)
    k_cache = np.asarray(k_cache, dtype=np.float32)
    v_cache = np.asarray(v_cache, dtype=np.float32)
    bt = np.asarray(block_tables, dtype=np.int32)

    assert q.shape == (NUM_SEQS * LQ, NUM_HEADS, HEAD_DIM)
    assert k_cache.shape == (TOTAL_BLOCKS, BLOCK_SIZE, NUM_KV_HEADS, HEAD_DIM)
    assert v_cache.shape == (TOTAL_BLOCKS, BLOCK_SIZE, NUM_KV_HEADS, HEAD_DIM)
    assert bt.shape == (NUM_SEQS, NBLK)
    assert bt.min() >= 0

    nc = _get_program()

    # DRAM row of logical kv token (s, t): block-table gather order
    t = np.arange(LK, dtype=np.int64)
    rows = np.concatenate(
        [bt[s, t // BLOCK_SIZE] * BLOCK_SIZE + t % BLOCK_SIZE
         for s in range(NUM_SEQS)])

    in_maps = [_marshal_core(q, k_cache, v_cache, rows, core)
               for core in range(NUM_KV_HEADS)]

    res = bass_utils.run_bass_kernel_spmd(
        nc, in_maps, core_ids=list(range(NUM_KV_HEADS)),
        trace=_want_trace,
        **({"trace_cores": list(range(NUM_KV_HEADS)), "stitch_traces": True}
           if _want_trace else {}),
    )

    out = np.empty((NUM_SEQS * LQ, NUM_HEADS, HEAD_DIM), dtype=np.float32)
    for core in range(NUM_KV_HEADS):
        out[:, core * GROUP:(core + 1) * GROUP, :] = res.results[core]["out"]

    if _want_trace:
        return out, res
    return out
